# revision 36
# baseline (speedup 1.0000x reference)
"""Trainium2 Bass kernel for nn_DeconvLayer (causal IIR filter).

Math: the reference IIR v[i] = x[i] + sum_j w[j] v[i-1-j] (i >= F, else 0)
has a geometrically-decaying impulse response h (|h[128]| ~ 1e-13), so it
equals a 128-tap causal FIR applied to x with the first F columns zeroed.

The device computes only the small CORRECTION  c = y - x = (h - delta) * xz
(xz = x with first F cols zeroed) as block-Toeplitz matmuls; the host adds
x back in fp32.  ||c|| ~ 0.18 ||y||, so both x and c travel in fp8 e4m3
(~2.7% RMS rounding) keeping end-to-end rel err ~1e-2 under the 2e-2 gate.

Design (v3):
  * ONE DoubleRow fp8 matmul per 512 output cols: stationary [128, 2, 128]
    = [A1 | A0], moving = overlapping 3D window [[256, 2], [1, 512]] over
    the time-blocked x (slot 0 = prev chunk, slot 1 = cur chunk).
    Streamed PE columns: 32768 (vs 65536 for the A0/A1 pair version).
  * Single persistent SBUF buffers (x 4.2MB, y 4.2MB); DRAM is laid out
    SLAB-CONTIGUOUS so every DMA descriptor is one large contiguous HBM
    read/write (no 33KB-stride hops).
  * Input slabs alternate between the two HWDGE rings (sync + scalar) so
    descriptor issue never throttles the stream; output stores ride the
    SWDGE (gpsimd) ring.
  * PSUM->SBUF fp32->fp8 drains split vector/scalar (the on-chip pacer at
    ~18.5us, under the ~23.6us DMA floor).

Layout trick: the host uploads x transposed AND 128-blocked as
[t, chunk, r] so time lands on the partition axis with no on-device
transposes; a leading halo chunk (prev core's last 128 steps, zeros for
core 0) makes every DoubleRow window contiguous.

Sharding: N = 131072 split into 8 column slabs of 16384, all B = 256
rows on every core.
"""

import sys

import numpy as np

if "/opt/trn_rl_repo" not in sys.path:
    sys.path.insert(0, "/opt/trn_rl_repo")

B = 256
N = 131072
F = 8
K = 128          # FIR taps == block size
P = 128          # partitions / block size
NCORES = 8
CORE_COLS = N // NCORES       # 16384 time steps per core
NCHUNK = CORE_COLS // P       # 128 chunks per core
FREE = B                      # free dim per chunk (batch rows)
XW = (NCHUNK + 1) * FREE      # x buffer width incl. halo chunk (33024)
YW = NCHUNK * FREE            # y buffer width (32768)
NPAIR = NCHUNK // 4           # 32 psum pair-tiles (1024 cols = 4 chunks each)

# slab splits (columns): small first so compute starts early, big middle
# for bandwidth, small last for a short pipeline tail
XSPLITS = [0, 1280, 2304, 4352, 8448, 12544, 16640, 20736, 24832, 28928, 33024]
YSPLITS = [0, 1024, 2048, 4096, 8192, 16384, 24576, 28672, 30720, 31744, 32768]

_CACHE = {}


def _impulse_response(w64):
    h = np.zeros(K, np.float64)
    h[0] = 1.0
    for n in range(1, K):
        acc = 0.0
        for j in range(min(F, n)):
            acc += w64[j] * h[n - 1 - j]
        h[n] = acc
    return h


def _toeplitz_mats(h):
    """A0[t, i] = h[i-t] for i > t (identity tap EXCLUDED -> correction);
    A1[t, i] = h[128+i-t] for t > i.  Returned in float64."""
    a0 = np.zeros((P, P), np.float64)
    a1 = np.zeros((P, P), np.float64)
    for t in range(P):
        for i in range(P):
            if i > t:
                a0[t, i] = h[i - t]
            elif t > i:
                a1[t, i] = h[K + i - t]
    return a0, a1


def _build_nc():
    from contextlib import ExitStack

    import concourse.mybir as mybir
    import concourse.tile as tile
    from concourse import bacc, bass

    f8 = mybir.dt.float8e4
    f32 = mybir.dt.float32

    nc = bacc.Bacc(
        "TRN2",
        target_bir_lowering=False,
        debug=False,
        enable_asserts=False,
        num_devices=NCORES,
    )
    # slab-contiguous flat DRAM tensors (each slab is one contiguous block)
    x_d = nc.dram_tensor("x_in", [1, P * XW], f8, kind="ExternalInput")
    w_d = nc.dram_tensor("wts", [P, 2, P], f8, kind="ExternalInput")
    y_out = nc.dram_tensor("y_out", [1, P * YW], f8, kind="ExternalOutput")

    with tile.TileContext(nc) as tc, ExitStack() as ctx:
        const = ctx.enter_context(tc.tile_pool(name="const", bufs=1))
        xpool = ctx.enter_context(tc.tile_pool(name="x", bufs=1))
        ypool = ctx.enter_context(tc.tile_pool(name="y", bufs=1))
        pspool = ctx.enter_context(tc.tile_pool(name="ps", bufs=4, space="PSUM"))

        wt = const.tile([P, 2, P], f8, tag="wt")
        # weights on the scalar HWDGE ring, in parallel with slab 0 on the
        # sync ring — both semaphores fire ~9.5us, unblocking the first MM
        nc.scalar.dma_start(wt[:, :, :], w_d[:, :, :])

        xbuf = xpool.tile([P, XW], f8)
        ybuf = ypool.tile([P, YW], f8)

        # input slabs: all on the sync HWDGE ring (scalar must stay free
        # for casts — a ring-full dma_start blocks its FIFO sequencer)
        for a, b in zip(XSPLITS[:-1], XSPLITS[1:]):
            w_cols = b - a
            src = bass.AP(x_d[:, :].tensor, a * P, [[w_cols, P], [1, w_cols]])
            nc.sync.dma_start(xbuf[:, a:b], src)

        # PE warm-up: a full HAM window (4096 cycles at 1.2GHz = 3.4us) of
        # back-to-back dummy matmuls (8 x 512 cols at 427ns cold) BEFORE the
        # data stream, so the clock un-throttle (1.2 -> 2.4GHz) fires
        # deterministically ~11us instead of lottery-style at 13-19us; the
        # data matmuls then run warm from the start and outpace the casts
        warm = const.tile([P, 512], f8, tag="warm")
        nc.gpsimd.memset(warm[:], 0.0)
        wps = pspool.tile([P, 1024], f32, name="ps_warm", tag="ps")
        for _ in range(8):
            nc.tensor.matmul(wps[:, :512], warm[:, :P], warm[:], start=True, stop=True)

        dr = mybir.MatmulPerfMode.DoubleRow
        ysi = 0  # next output slab to store
        for p in range(NPAIR):
            ps = pspool.tile([P, 1024], f32, name=f"ps_{p}", tag="ps")
            for h in range(2):
                # out chunks (c, c+1); window starts one chunk earlier in
                # xbuf (whose chunk 0 is the halo): offset = c*256, pair
                # slot 0 = prev chunk, slot 1 = cur chunk (overlapping)
                c = 4 * p + 2 * h
                base = xbuf[:, c * FREE : c * FREE + 3 * FREE]
                rhs = bass.AP(
                    base.tensor,
                    base.offset,
                    [list(base.ap[0]), [FREE, 2], [1, 512]],
                )
                nc.tensor.matmul(
                    ps[:, h * 512 : (h + 1) * 512],
                    wt[:, :, :],
                    rhs,
                    start=True,
                    stop=True,
                    perf_mode=dr,
                )
            # PSUM->SBUF drain with fp32 -> e4m3 cast, alternating engines
            # in pair-arrival order (scalar takes the last pair — ACTIVATE
            # is ~9% faster so the tail is shortest on it)
            dst = ybuf[:, p * 1024 : (p + 1) * 1024]
            if p % 2 == 1:
                nc.scalar.copy(dst, ps[:])
            else:
                nc.vector.tensor_copy(dst, ps[:])

            # output stores on the SWDGE (gpsimd) ring, slab-contiguous;
            # the FINAL store rides the (idle by then) sync HWDGE ring so
            # the last two stores issue in parallel instead of serializing
            # on gpsimd's ~0.65us per-issue cost
            done_cols = (p + 1) * 1024
            while ysi < len(YSPLITS) - 1 and YSPLITS[ysi + 1] <= done_cols:
                a, b = YSPLITS[ysi], YSPLITS[ysi + 1]
                w_cols = b - a
                dst_d = bass.AP(
                    y_out[:, :].tensor, a * P, [[w_cols, P], [1, w_cols]]
                )
                eng = nc.sync if ysi == len(YSPLITS) - 2 else nc.gpsimd
                eng.dma_start(dst_d, ybuf[:, a:b])
                ysi += 1
    nc.compile()
    return nc


def _get_nc():
    if "nc" not in _CACHE:
        _CACHE["nc"] = _build_nc()
    return _CACHE["nc"]


LAST_RESULTS = None


def kernel(x, w=None, _trace=False, **_ignored):
    global LAST_RESULTS
    import ml_dtypes
    from concourse.bass_utils import run_bass_kernel_spmd

    f8 = ml_dtypes.float8_e4m3

    x = np.asarray(x, dtype=np.float32)
    assert x.shape == (B, N)
    if w is None:
        import jax
        import jax.numpy as jnp

        key = jax.random.key(0)
        _, k2 = jax.random.split(key)
        w = np.asarray(jax.random.normal(k2, (F,), dtype=jnp.float32) * 0.05)
    w = np.asarray(w, dtype=np.float32)

    h = _impulse_response(w.astype(np.float64))
    a0, a1 = _toeplitz_mats(h)
    # DoubleRow stationary: slot 0 pairs with the earlier (prev-chunk)
    # window -> A1; slot 1 with the current chunk -> A0
    wpack = np.stack([a1, a0], axis=1).astype(f8)  # [128, 2, 128]

    # transposed, 128-blocked input: [t, chunk, r]
    xt = np.array(x.T)  # [N, B]
    xt[:F] = 0.0  # v[i] = 0 for i < F
    xb = np.ascontiguousarray(
        xt.reshape(NCORES * NCHUNK, P, B).transpose(1, 0, 2)
    ).astype(f8)  # [128, 1024, 256]
    zhalo = np.zeros((P, B), f8)

    in_maps = []
    for c in range(NCORES):
        lo_c = c * NCHUNK
        halo = zhalo[:, None, :] if c == 0 else xb[:, lo_c - 1 : lo_c, :]
        xc = np.concatenate([halo, xb[:, lo_c : lo_c + NCHUNK, :]], axis=1)
        xc = np.ascontiguousarray(xc).reshape(P, XW)
        # slab-contiguous flat layout
        xflat = np.concatenate(
            [xc[:, a:b].ravel() for a, b in zip(XSPLITS[:-1], XSPLITS[1:])]
        )[None, :]
        in_maps.append({"x_in": xflat, "wts": wpack})

    nc = _get_nc()
    res = run_bass_kernel_spmd(
        nc, in_maps, core_ids=list(range(NCORES)), trace=_trace
    )
    LAST_RESULTS = res
    # reassemble: per core flat slabs -> [128, YW] -> [NCHUNK, P, B]
    parts = []
    for r in res.results:
        yflat = np.asarray(r["y_out"]).reshape(-1)
        cb = np.empty((P, YW), np.float32)
        for a, b in zip(YSPLITS[:-1], YSPLITS[1:]):
            cb[:, a:b] = (
                yflat[a * P : b * P].reshape(P, b - a).astype(np.float32)
            )
        cb = cb.reshape(P, NCHUNK, B).transpose(1, 0, 2)
        parts.append(cb.reshape(CORE_COLS, B))
    ct = np.concatenate(parts, axis=0)  # correction, [N, B]
    y = x + np.ascontiguousarray(ct.T)  # add identity tap back in fp32
    y[:, :F] = 0.0  # reference zeroes the first F steps
    return y


if __name__ == "__main__":
    rng = np.random.default_rng(0)
    x = rng.standard_normal((B, N), dtype=np.float32)
    w = (rng.standard_normal(F) * 0.05).astype(np.float32)
    y = kernel(x, w)
    print("kernel ran, y shape:", y.shape)
